# revision 1
# baseline (speedup 1.0000x reference)
"""Trainium2 Bass kernel for the DichotomicSolver problem.

Problem: x [4096, 2048] f32; 19 iterations of soft bisection per row:
    m_new = active ? (lb+ub)/2 : m
    Dm    = mean_s sigmoid(K*(m_new - x[:, s])) - 0.5
    H     = sigmoid(K*Dm)
    lb,ub soft-update (interval halves exactly); active &= |Dm| >= STEP
Output: m [4096, 1].

Sharding: pure data parallel - 512 rows per core on 8 cores, no
cross-core communication. Each core keeps its 4MB x shard resident in
SBUF (loaded once) and runs the whole solve on-chip; x is read from HBM
exactly once (memory-optimal).

Per-core layout: 4 row-tiles of [128, 2048] (batch in partitions).
Each heavy iteration issues one ACTIVATE per row-tile computing
sigmoid(-K*x + cK) with a per-partition bias and a fused free-dim
row-sum (accum_out). The whole recurrence critical path lives on the
scalar engine (sigmoid passes -> H -> midpoint update -> next biases,
all same-engine, no cross-engine semaphore hops); the vector engine
only maintains the found/active bookkeeping off the critical path.

Structure vs the reference (same per-row trajectory):
 - state is tracked scaled by K: cK = K*c (midpoint). The ACT bias for
   sigmoid(K*(c - x)) = sigmoid(-K*x + cK) is the state itself. The
   sigmoid pass is evaluated with the *unfrozen* midpoint for every
   row; frozen rows' results are simply never consumed, because the
   active mask (explicitly AND-accumulated on the vector engine) gates
   the output select. Output is m = mKout/K.
 - interval half-width is deterministic: r_i = 50/2^i, a compile-time
   constant, so the lb/ub pair reduces to the midpoint with
   cK' = cK + (0.5 - H)*K*r_i = (-K*r_i)*H + (cK + K*r_i/2), evaluated
   on the scalar engine as one Identity activation per row-tile
   (scale=-K*r_i, bias=cksh precomputed off-path). Identity is in the
   same ACT table set as Sigmoid - no table switches.
 - H = sigmoid(K*(ssum/S - 0.5)) is evaluated directly from the row
   sum (bias=-K/2, scale=K/S; 1/S is a power of two so Dm's rounding
   matches the reference mean).
 - the width condition (ub-lb > 2*STEP) can never fire within the 19
   iterations (width at iter 18 is 3.81e-4 > 2e-4, exact halving).
 - iteration 19 only consumes the m-update; its Dm/H/c are dead, so
   only 18 sigmoid passes are issued.
"""

import numpy as np

import concourse.bacc as bacc
import concourse.mybir as mybir
import concourse.tile as tile
from concourse.bass_utils import run_bass_kernel_spmd

N_CORES = 8
BS, S = 4096, 2048
ROWS = BS // N_CORES  # 512 rows per core
P = 128
NT = ROWS // P  # 4 row-tiles per core

K = 30.0
STEP = 1e-4
HALF0 = 50.0  # (UB - LB) / 2
N_ITERS = 19  # ceil(log2((UB-LB)/(2*STEP)))
STEP2 = float(np.float32(STEP) * np.float32(STEP))
F32 = mybir.dt.float32
Sigmoid = mybir.ActivationFunctionType.Sigmoid
Identity = mybir.ActivationFunctionType.Identity
Op = mybir.AluOpType


def _emit(tc, out_ap, x_ap, reps=1):
    nc = tc.nc

    with (
        tc.tile_pool(name="xres", bufs=1) as xpool,
        tc.tile_pool(name="state", bufs=1) as st,
    ):
        # x resident in SBUF: 4 x [128, 2048] f32 = 32KB/partition.
        xt = []
        for t in range(NT):
            xtile = xpool.tile([P, S], F32, tag=f"x{t}", name=f"x{t}")
            nc.sync.dma_start(out=xtile[:], in_=x_ap[t * P : (t + 1) * P, :])
            xt.append(xtile)

        # Sigmoid output sink (values unused; only accum_out matters).
        # Full-width SBUF stores; a step-0 broadcast dummy out measures
        # ~6us slower in the full kernel despite winning in isolation.
        sig = [
            xpool.tile([P, S], F32, tag=f"sig{k}", name=f"sig{k}") for k in range(2)
        ]

        # State, column t = row-tile t.
        def stt(name, dtype=F32):
            return st.tile([P, NT], dtype, tag=name, name=name)

        ck = stt("ck")      # K * midpoint (always-updated, never frozen)
        cksh = stt("cksh")  # cK + K*r_i/2
        ssum = stt("ssum")  # row sums of sigmoid
        dm = stt("dm")      # Dm
        sq = stt("sq")      # Dm^2
        h = stt("h")        # H
        nf = stt("nf")      # not-found mask (1.0/0.0)
        tq = stt("tq")      # (-K*r)*H scratch
        act = stt("act")    # active mask (1.0/0.0), AND-accumulated
        mko = stt("mko")    # K * m (frozen via act-gated select)
        mout = stt("mout")  # final m
        bm15 = st.tile([P, 1], F32, tag="bm15", name="bm15")  # const -K/2
        nc.vector.memset(bm15[:], -K / 2)

        def act_pass():
            for t in range(NT):
                nc.scalar.activation(
                    out=sig[t % 2][:],
                    in_=xt[t][:],
                    func=Sigmoid,
                    bias=ck[:, t : t + 1],
                    scale=-K,
                    accum_out=ssum[:, t : t + 1],
                )

        def solve():
            # Iteration 0: all rows active; m = c = 50.
            nc.vector.memset(ck[:], K * HALF0)
            nc.vector.memset(cksh[:], K * HALF0 * 1.5)  # cK_0 + K*r_0/2
            nc.vector.memset(act[:], 1.0)
            nc.vector.tensor_copy(out=mko[:], in_=ck[:])
            act_pass()

            # Heavy iterations i = 0..17: consume ssum_i, produce
            # cK_{i+1}, active_{i+1}, mK_{i+1}; issue iteration i+1's
            # sigmoid pass (i < 17).
            r = HALF0
            for i in range(N_ITERS - 1):
                # H = sigmoid(K*(ssum/S - 0.5)) straight from ssum (ACT).
                nc.scalar.activation(
                    h[:], ssum[:], Sigmoid, bias=bm15[:, 0:1], scale=K / S
                )
                # cK' = (-K*r)*H + cksh, one Identity ACTIVATE per
                # row-tile (per-partition bias) - still on ACT, so the
                # next sigmoid pass needs no cross-engine wait. (A DVE
                # version of this update measures slower: the
                # ACT->DVE->ACT semaphore round trip costs more than
                # the four extra small ACT ops.)
                # off the critical path (vector engine), emitted BEFORE
                # the next sigmoid passes so their ssum overwrites order
                # after these reads (WAR): Dm = ssum/S - 0.5 (1/S power
                # of two: exact), then nf = Dm^2 >= STEP^2 (== |Dm| >= STEP)
                nc.vector.tensor_scalar(
                    dm[:], ssum[:], 1.0 / S, 0.5, Op.mult, Op.subtract
                )
                nc.vector.tensor_mul(sq[:], dm[:], dm[:])
                nc.vector.tensor_scalar(nf[:], sq[:], STEP2, None, Op.is_ge)
                last = i >= N_ITERS - 2
                for t in range(NT):
                    nc.scalar.activation(
                        ck[:, t : t + 1], h[:, t : t + 1], Identity,
                        bias=cksh[:, t : t + 1], scale=-K * r,
                    )
                if not last:
                    act_pass()
                # active_{i+1} = active_i & nf_i
                nc.vector.tensor_mul(act[:], act[:], nf[:])
                # mK_{i+1} = active_{i+1} ? cK_{i+1} : mK_i
                nc.vector.copy_predicated(
                    out=mko[:], mask=act[:].bitcast(mybir.dt.uint32), data=ck[:]
                )
                if not last:
                    # cksh_{i+1} = cK' + K*r_{i+1}/2 (read by next ck
                    # update's bias - ready well before needed)
                    nc.vector.tensor_scalar_add(cksh[:], ck[:], K * r * 0.25)
                r *= 0.5

        if reps == 1:
            solve()
        else:
            # benchmark mode: repeat the solve in a hardware loop so the
            # per-solve time can be extracted as a slope over reps,
            # cancelling NEFF launch / RPC overheads. Warm the sigmoid
            # table set outside the loop first.
            nc.scalar.activation(h[:], ck[:], Sigmoid, bias=bm15[:, 0:1], scale=1.0)
            with tc.For_i(0, reps, 1):
                solve()

        # out = mK / K
        nc.vector.tensor_scalar_mul(mout[:], mko[:], 1.0 / K)
        for t in range(NT):
            nc.sync.dma_start(
                out=out_ap[t * P : (t + 1) * P, :], in_=mout[:, t : t + 1]
            )


_NC_CACHE = {}


def _build(reps=1):
    if reps in _NC_CACHE:
        return _NC_CACHE[reps]
    nc = bacc.Bacc(
        "TRN2",
        target_bir_lowering=False,
        debug=False,
        enable_asserts=False,
        num_devices=N_CORES,
    )
    x_ap = nc.dram_tensor("x", [ROWS, S], F32, kind="ExternalInput").ap()
    out_ap = nc.dram_tensor("out", [ROWS, 1], F32, kind="ExternalOutput").ap()
    with tile.TileContext(nc) as tc:
        _emit(tc, out_ap, x_ap, reps=reps)
    nc.compile()
    _NC_CACHE[reps] = nc
    return nc


def run(x, trace=False, **spmd_kwargs):
    """Run on 8 NeuronCores. x: [4096, 2048] f32. Returns (out, results)."""
    assert x.shape == (BS, S), x.shape
    nc = _build()
    x = np.ascontiguousarray(x, dtype=np.float32)
    in_maps = [{"x": x[c * ROWS : (c + 1) * ROWS]} for c in range(N_CORES)]
    last_exc = None
    for attempt in range(3):
        try:
            res = run_bass_kernel_spmd(
                nc, in_maps, core_ids=list(range(N_CORES)), trace=trace,
                **spmd_kwargs,
            )
            break
        except Exception as e:  # transient axon-worker wedges recover on retry
            last_exc = e
            import time as _time

            _time.sleep(10 * (attempt + 1))
    else:
        raise last_exc
    out = np.concatenate([res.results[c]["out"] for c in range(N_CORES)], axis=0)
    return out, res


def kernel(x):
    out, _ = run(np.asarray(x))
    return out

